# revision 5
# baseline (speedup 1.0000x reference)
"""Linear Recurrent Unit (dense transition) on 8 Trainium2 NeuronCores.

h_t = A h_{t-1} + (B x_t + c),  A = 0.9 I + 0.1 A_raw (fixed), T = 8192.

Strategy (sequence parallelism over T, per the sharding hint):
  * T is split into 8 contiguous shards of 1024 steps, one per core.
  * Launch A (per core): b = B x^T + c via matmul, then chunk totals
    u1[k] = sum_{r<8} A^{7-r} b[8k+r] via 8 accumulating matmuls with
    host-precomputed (A^d)^T weight tiles. Outputs b and u1.
  * Host: exact fp64 scan over the 1024 chunk carries (8 cores x 128
    chunks, O(T/c * H^2) ~ 4 MFLOP -- the "small cross-device scan over
    per-shard carries") -> per-chunk seed states s1.
  * Launch B (per core): h[8k+r] = A^{r+1} s1[k]
      + sum_{p=1..7} A^p b[8k+r-p] + b[8k+r]
    via 16 seed matmuls + 14 diagonal matmuls (strided access patterns)
    accumulated in PSUM, final add of b on the vector engine. Outputs h^T.
  * Params (A powers, B, c) are replicated to every core.

Inputs cross the host between the two launches only as the per-shard
b/u1/s1 buffers; all O(T*H*X) compute and O(T) data movement is on-device.
"""

import numpy as np

import concourse.bacc as bacc
import concourse.mybir as mybir
import concourse.tile as tile
from concourse.bass_utils import run_bass_kernel_spmd

H = 64
X = 128
T = 8192
NC = 8
TL = T // NC          # 1024 timesteps per core
C1 = 8                # level-1 chunk length
K1 = TL // C1         # 128 chunks per core
A_SCALE = 0.1
A_IDENTITY = 0.9

F32 = mybir.dt.float32
# Matmul operand dtype: float32 (exact, 2 cyc/col) or float32r (~1e-4, 1 cyc/col)
DT_MM = mybir.dt.float32

_programs = {}


def _build_prog_a(dt_mm):
    """Launch A: xT -> b (=B x + c) and chunk totals u1."""
    nc = bacc.Bacc("TRN2", target_bir_lowering=False, debug=False, num_devices=NC)
    xT_d = nc.dram_tensor("xT", [X, TL], dt_mm, kind="ExternalInput")
    wb_d = nc.dram_tensor("wb", [X, H], dt_mm, kind="ExternalInput")      # B^T
    pw_d = nc.dram_tensor("pw", [H, C1 * H], dt_mm, kind="ExternalInput")  # (A^d)^T d=0..7
    c_d = nc.dram_tensor("cvec", [H, 1], F32, kind="ExternalInput")
    b_out = nc.dram_tensor("b_out", [H, TL], dt_mm, kind="ExternalOutput")
    u1_out = nc.dram_tensor("u1_out", [H, K1], F32, kind="ExternalOutput")

    with tile.TileContext(nc) as tc:
        with (
            tc.tile_pool(name="sbuf", bufs=1) as sbuf,
            tc.tile_pool(name="psum", bufs=1, space="PSUM") as psum,
        ):
            xT = sbuf.tile([X, TL], dt_mm, tag="xT")
            wb = sbuf.tile([X, H], dt_mm, tag="wb")
            pw = sbuf.tile([H, C1 * H], dt_mm, tag="pw")
            cv = sbuf.tile([H, 1], F32, tag="cv")
            nc.sync.dma_start(wb[:], wb_d[:])
            nc.sync.dma_start(pw[:], pw_d[:])
            nc.sync.dma_start(cv[:], c_d[:])
            nc.sync.dma_start(xT[:], xT_d[:])

            b_ps = psum.tile([H, TL], F32, tag="b_ps")
            for hf in range(2):
                cols = slice(hf * 512, hf * 512 + 512)
                nc.tensor.matmul(b_ps[:, cols], wb[:], xT[:, cols])

            b_sb = sbuf.tile([H, TL], dt_mm, tag="b_sb")
            for hf in range(2):
                cols = slice(hf * 512, hf * 512 + 512)
                nc.vector.tensor_scalar_add(b_sb[:, cols], b_ps[:, cols], cv[:])

            u1_ps = psum.tile([H, K1], F32, tag="u1_ps")
            b_v = b_sb[:].rearrange("h (k r) -> h k r", r=C1)
            for dd in range(C1):
                nc.tensor.matmul(
                    u1_ps[:],
                    pw[:, dd * H:(dd + 1) * H],
                    b_v[:, :, C1 - 1 - dd],
                    start=(dd == 0), stop=(dd == C1 - 1),
                )
            u1_sb = sbuf.tile([H, K1], F32, tag="u1_sb")
            nc.vector.tensor_copy(u1_sb[:], u1_ps[:])

            nc.sync.dma_start(b_out[:], b_sb[:])
            nc.sync.dma_start(u1_out[:], u1_sb[:])
    nc.compile()
    return nc


def _build_prog_b(dt_mm):
    """Launch B: b + chunk seeds s1 -> h^T."""
    nc = bacc.Bacc("TRN2", target_bir_lowering=False, debug=False, num_devices=NC)
    b_d = nc.dram_tensor("b_in", [H, TL], dt_mm, kind="ExternalInput")
    s1_d = nc.dram_tensor("s1_in", [H, K1], dt_mm, kind="ExternalInput")
    pw_d = nc.dram_tensor("pw", [H, (C1 + 1) * H], dt_mm, kind="ExternalInput")  # (A^d)^T d=0..8
    h_out = nc.dram_tensor("hT_out", [H, TL], F32, kind="ExternalOutput")

    with tile.TileContext(nc) as tc:
        with (
            tc.tile_pool(name="sbuf", bufs=1) as sbuf,
            tc.tile_pool(name="psum", bufs=1, space="PSUM") as psum,
        ):
            b_sb = sbuf.tile([H, TL], dt_mm, tag="b_sb")
            s1 = sbuf.tile([H, K1], dt_mm, tag="s1")
            pw = sbuf.tile([H, (C1 + 1) * H], dt_mm, tag="pw")
            nc.sync.dma_start(pw[:], pw_d[:])
            nc.sync.dma_start(s1[:], s1_d[:])
            nc.sync.dma_start(b_sb[:], b_d[:])

            # h_ps holds, per bank half hf (chunks hf*64..hf*64+63), the
            # r-major layout: psum col hf*512 + r*64 + k  <->  time 8k+r.
            # r-major makes every matmul OUT contiguous (sim requires <=1
            # free dim on matmul outputs); rhs views are 3D strided.
            h_ps = psum.tile([H, TL], F32, tag="h_ps")
            # b viewed as [h, kk(half), r, k]: element offset kk*512 + k*8 + r
            b_rk = b_sb[:].rearrange("h (kk k r) -> h kk r k", kk=2, r=C1)
            # h_ps viewed as [h, kk, k, r] for the natural-order DVE read
            h_kr = h_ps[:].rearrange("h (kk r k) -> h kk k r", kk=2, r=C1)

            KH = K1 // 2  # 64 chunks per PSUM bank half
            for hf in range(2):
                cols = slice(hf * 512, hf * 512 + 512)
                # contiguous starter: h(r,k) = I @ b(k,r)  (the p=0 term)
                nc.tensor.matmul(
                    h_ps[:, cols], pw[:, 0:H], b_rk[:, hf, :, :],
                    start=True, stop=False,
                )
                # seeds: h[:, (r,k)] += A^{r+1} s1[:, k]
                for r in range(C1):
                    nc.tensor.matmul(
                        h_ps[:, hf * 512 + r * KH: hf * 512 + (r + 1) * KH],
                        pw[:, (r + 1) * H:(r + 2) * H],
                        s1[:, hf * KH:(hf + 1) * KH],
                        start=False, stop=False,
                    )
                # diagonals p=7..1: h[:, (r,k)] += A^p b[:, (k, r-p)], r >= p
                for p in range(C1 - 1, 0, -1):
                    nc.tensor.matmul(
                        h_ps[:, hf * 512 + p * KH: hf * 512 + 512],
                        pw[:, p * H:(p + 1) * H],
                        b_rk[:, hf, 0:C1 - p, :],
                        start=False, stop=(p == 1),
                    )

            # PSUM -> SBUF copy that also restores natural k-major order
            h_sb = sbuf.tile([H, TL], F32, tag="h_sb")
            h_sb_v = h_sb[:].rearrange("h (kk k r) -> h kk k r", kk=2, r=C1)
            for hf in range(2):
                nc.vector.tensor_copy(h_sb_v[:, hf, :, :], h_kr[:, hf, :, :])
            nc.sync.dma_start(h_out[:], h_sb[:])
    nc.compile()
    return nc


def _get_programs():
    key = str(DT_MM)
    if key not in _programs:
        _programs[key] = (_build_prog_a(DT_MM), _build_prog_b(DT_MM))
    return _programs[key]


def _prep(x_seq, h0, A_raw, B, c):
    """Host-side shard + replicated weights (fp64 matrix powers)."""
    A = (A_IDENTITY * np.eye(H) + A_SCALE * A_raw).astype(np.float64)
    pows = [np.eye(H)]
    for _ in range(C1):
        pows.append(A @ pows[-1])
    powT_a = np.concatenate([p.T for p in pows[:C1]], axis=1).astype(np.float32)
    powT_b = np.concatenate([p.T for p in pows], axis=1).astype(np.float32)
    wb = np.ascontiguousarray(B.T).astype(np.float32)             # [X, H]
    cv = c.reshape(H, 1).astype(np.float32)
    xTs = [np.ascontiguousarray(x_seq[i * TL:(i + 1) * TL].T).astype(np.float32)
           for i in range(NC)]
    return A, pows, powT_a, powT_b, wb, cv, xTs


def _host_carry_scan(u1s, h0, A, pows):
    """fp64 scan over the 8*128 chunk carries -> per-chunk seed states."""
    A8 = pows[C1]
    s = h0.astype(np.float64).copy()
    s1s = []
    for i in range(NC):
        u1 = u1s[i].astype(np.float64)
        s1 = np.empty((H, K1), np.float64)
        for k in range(K1):
            s1[:, k] = s
            s = A8 @ s + u1[:, k]
        s1s.append(s1.astype(np.float32))
    return s1s


def kernel(x_seq, h0, A_raw, B, c, _trace=False):
    prog_a, prog_b = _get_programs()
    A, pows, powT_a, powT_b, wb, cv, xTs = _prep(x_seq, h0, A_raw, B, c)
    cores = list(range(NC))

    in_a = [{"xT": xTs[i], "wb": wb, "pw": powT_a, "cvec": cv} for i in range(NC)]
    res_a = run_bass_kernel_spmd(prog_a, in_a, cores, trace=_trace,
                                 trace_cores=cores if _trace else None)
    u1s = [res_a.results[i]["u1_out"] for i in range(NC)]
    bs = [res_a.results[i]["b_out"] for i in range(NC)]

    s1s = _host_carry_scan(u1s, h0, A, pows)

    in_b = [{"b_in": bs[i], "s1_in": s1s[i], "pw": powT_b} for i in range(NC)]
    res_b = run_bass_kernel_spmd(prog_b, in_b, cores, trace=_trace,
                                 trace_cores=cores if _trace else None)

    h = np.empty((T, H), np.float32)
    for i in range(NC):
        h[i * TL:(i + 1) * TL] = res_b.results[i]["hT_out"].T
    if _trace:
        return h, (res_a, res_b)
    return h
